# revision 10
# baseline (speedup 1.0000x reference)
"""Trainium2 Bass kernel: per-pixel 5x5 kernel application (KPN-style).

    out[b,c,y,x] = sum_{i,j} softmax(kernels[b,:,y,x])[i*5+j]
                   * zpad(data)[b,c,y+i,x+j]          (i,j in 0..4, r=2)

Sharding (8 NeuronCores, pure data parallel, no collectives):
    core = (b, H-half): 4 batches x 2 row-bands of 360 rows.
    Halo rows come from overlapping host-side slices of the full input.

Per-core algorithm (tiles live in "data space": 124 partitions =
120 output rows + 2 halo rows each side):
    - unnormalized softmax: E_t = exp(K_t) on ACT (bf16), denominator
      accumulated on the PE together with the taps.
    - DVE forms per-tap product planes Q_{t,c} = E_t * D_c (bf16, 2x mode);
      x-shift dj is a free-dim offset (two parity-aligned bf16 copies of the
      data keep operands 4-byte aligned); y-shift di is folded into the
      kernel-tensor DMA (rows loaded shifted by -di) and undone by the PE's
      stationary shift matrix S_di[k,m] = [k == m+di].
    - PE accumulates the 25 tap planes (and the 25 exp planes) into PSUM
      with shifted-identity matmuls.
    - DVE: out_c = PSUM_c * reciprocal(PSUM_sum).

DMA layout notes: kernel-tensor loads are one DMA per tap plane so the
DRAM side is a single contiguous region (strided small-chunk patterns get
pinned to a few SDMA engines); inputs are pre-converted to bf16 on the
host to halve HBM traffic; loads alternate between the two HWDGE rings
(sync + scalar); the output is staged full-width and stored once per
row-tile.

kernel(**inputs) takes the FULL inputs and returns the FULL output.
"""

import numpy as np
import ml_dtypes

B, C, H, W, KW = 4, 3, 720, 1280, 5
NCORES = 8
HS = H // 2            # 360 output rows per shard
RT = 120               # output rows per row-tile
NRT = HS // RT         # 3 row-tiles
HALO = 2
DP = RT + 2 * HALO     # 124 partitions (data space)
WP = 1288              # padded data width: 2 left + 1280 + 6 right
KROWPAD = 4            # zero rows around each kernel shard (top+bottom)
KH = HS + 2 * KROWPAD  # 368
XCH = [(0, 512), (512, 512), (1024, 256)]

KERN_BF16 = True       # ship kernels to HBM as bf16 (halves DMA traffic)

_CACHE = {}


def _build_program():
    import concourse.bacc as bacc
    import concourse.mybir as mybir
    from concourse.bass import AP
    from concourse import tile

    f32 = mybir.dt.float32
    bf16 = mybir.dt.bfloat16
    kdt = bf16 if KERN_BF16 else f32

    nc = bacc.Bacc(
        "TRN2",
        target_bir_lowering=False,
        debug=False,
        enable_asserts=False,
        num_devices=NCORES,
    )
    # Row-major host layouts: one large contiguous DMA descriptor per
    # SBUF partition (small strided descriptors throttle the SDMA engines).
    d_data = nc.dram_tensor("data", [HS + 2 * HALO, C, WP], bf16, kind="ExternalInput")
    d_kern = nc.dram_tensor("kern", [KH, KW * KW, W], kdt, kind="ExternalInput")
    d_out = nc.dram_tensor("out", [HS, C, W], f32, kind="ExternalOutput")

    # Shift matrices S_di[k, m] = 1 iff k == m + di  (k: 124 data rows,
    # m: 120 out rows). Baked into the NEFF as a Const tensor.
    s_np = np.zeros((KW, DP, RT), dtype=ml_dtypes.bfloat16)
    for di in range(KW):
        for m in range(RT):
            s_np[di, m + di, m] = 1.0
    d_s = nc.inline_tensor(np.ascontiguousarray(s_np), "smat")

    KROW = KW * KW * W  # element stride between rows of d_kern

    with tile.TileContext(nc) as tc:
        with tc.tile_pool(name="const", bufs=1) as cpool, \
             tc.tile_pool(name="dbf", bufs=2) as dbfpool, \
             tc.tile_pool(name="kt", bufs=3) as kpool, \
             tc.tile_pool(name="et", bufs=7) as epool, \
             tc.tile_pool(name="qt", bufs=4) as qpool, \
             tc.tile_pool(name="fin", bufs=2) as fpool, \
             tc.tile_pool(name="ps", bufs=2, space="PSUM") as ppool:

            s_sb = cpool.tile([DP, KW, RT], bf16)
            nc.sync.dma_start(out=s_sb[:], in_=d_s.ap().transpose([1, 0, 2]))

            for rt in range(NRT):
                y0 = rt * RT

                # data rows y0 .. y0+123 of the (row-padded) shard, bf16.
                # dbf1 is shifted one element left so odd-dj slices stay
                # 4-byte aligned (keeps DVE 2x mode).
                # kernel taps: one SWDGE DMA per di-group (5 tap planes,
                # rows shifted by -di) — SWDGE spreads descriptors across
                # all 16 SDMA engines (HWDGE pins them to 4); exp per group.
                # dbf0 also rides SWDGE (issued after the first kt so the
                # first exp is gated only by a 124-descriptor load); dbf1
                # (the 1-element x-shifted copy for odd-dj alignment) is a
                # cheap DVE bf16 copy, not a DMA, so no DMA->DMA dep chain.
                dbf0 = dbfpool.tile([DP, C, WP], bf16, tag="dbf0")
                dbf1 = dbfpool.tile([DP, C, WP], bf16, tag="dbf1")
                # byte-balance the three DMA paths (SWDGE sustains ~70GB/s
                # of descriptor flow; the two HWDGE rings share SDMA engines
                # 0-3 for ~110GB/s combined): di 0,1 + data on SWDGE,
                # di 2,3 on the sync ring, di 4 (+ the store) on scalar.
                ets = []
                kt_eng = [nc.gpsimd, nc.gpsimd, nc.sync, nc.sync, nc.scalar]
                for di in range(KW):
                    kt = kpool.tile([DP, KW, W], kdt, tag="kt")
                    off = (KROWPAD + y0 - di) * KROW + di * KW * W
                    kt_eng[di].dma_start(
                        out=kt[:],
                        in_=AP(d_kern, off, [[KROW, DP], [W, KW], [1, W]]),
                    )
                    if di == 0:
                        nc.gpsimd.dma_start(
                            out=dbf0[:], in_=d_data.ap()[y0:y0 + DP],
                        )
                    et = epool.tile([DP, KW, W], bf16, tag="et")
                    nc.scalar.activation(
                        et[:], kt[:], mybir.ActivationFunctionType.Exp,
                    )
                    ets.append(et)
                f0 = dbf0[:].rearrange("p c w -> p (c w)")
                f1 = dbf1[:].rearrange("p c w -> p (c w)")
                nc.vector.tensor_copy(f1[:, 0:C * WP - 1], f0[:, 1:C * WP])

                rs = fpool.tile([RT, W], f32, tag="rs", bufs=1)
                ost = fpool.tile([RT, C, W], f32, tag="ost")

                for (xc, xcw) in XCH:
                    # PSUM banks: 0..2 = channel accumulators, 3 = sumexp
                    pacc = ppool.tile([RT, 4, 512], f32, tag="pacc")

                    for di in range(KW):
                        et = ets[di]
                        lhs = s_sb[:, di, :]
                        first = di == 0
                        last = di == KW - 1
                        for dj in range(KW):
                            nc.tensor.matmul(
                                out=pacc[:, 3, 0:xcw],
                                lhsT=lhs,
                                rhs=et[:, dj, xc:xc + xcw],
                                start=first and dj == 0,
                                stop=last and dj == KW - 1,
                            )
                        for dj in range(KW):
                            qt = qpool.tile([DP, C, 512], bf16, tag="qt")
                            if dj % 2 == 0:
                                dsrc = dbf0[:, :, xc + dj:xc + dj + xcw]
                            else:
                                dsrc = dbf1[:, :, xc + dj - 1:xc + dj - 1 + xcw]
                            esrc = (
                                et[:, dj, xc:xc + xcw]
                                .unsqueeze(1)
                                .broadcast_to([DP, C, xcw])
                            )
                            nc.vector.tensor_tensor(
                                qt[:, :, 0:xcw], esrc, dsrc, mybir.AluOpType.mult,
                            )
                            for c in range(C):
                                nc.tensor.matmul(
                                    out=pacc[:, c, 0:xcw],
                                    lhsT=lhs,
                                    rhs=qt[:, c, 0:xcw],
                                    start=first and dj == 0,
                                    stop=last and dj == KW - 1,
                                )

                    nc.vector.reciprocal(rs[:, xc:xc + xcw], pacc[:, 3, 0:xcw])
                    rsb = (
                        rs[:, xc:xc + xcw].unsqueeze(1).broadcast_to([RT, C, xcw])
                    )
                    nc.vector.tensor_tensor(
                        ost[:, :, xc:xc + xcw], pacc[:, 0:3, 0:xcw], rsb,
                        mybir.AluOpType.mult,
                    )

                nc.scalar.dma_start(out=d_out.ap()[y0:y0 + RT], in_=ost[:])

    nc.compile()
    return nc


def get_program():
    if "nc" not in _CACHE:
        _CACHE["nc"] = _build_program()
    return _CACHE["nc"]


def make_shards(data: np.ndarray, kernels: np.ndarray):
    """Full inputs -> per-core input maps (with halo + zero padding)."""
    data = np.asarray(data, dtype=np.float32)
    kernels = np.asarray(kernels, dtype=np.float32)
    kdt = ml_dtypes.bfloat16 if KERN_BF16 else np.float32
    # zero-pad data: 2 rows top/bottom, 2 cols left, 6 cols right;
    # row-major layouts: data [row, c, x], kern [row, tap, x]
    dpad = np.zeros((B, H + 2 * HALO, C, WP), dtype=ml_dtypes.bfloat16)
    dpad[:, HALO:HALO + H, :, HALO:HALO + W] = (
        data.transpose(0, 2, 1, 3).astype(ml_dtypes.bfloat16)
    )
    in_maps = []
    for core in range(NCORES):
        b, hh = divmod(core, 2)
        r0 = hh * HS
        dsh = np.ascontiguousarray(dpad[b, r0:r0 + HS + 2 * HALO])
        ksh = np.zeros((KH, KW * KW, W), dtype=kdt)
        ksh[KROWPAD:KROWPAD + HS] = (
            kernels[b, :, r0:r0 + HS, :].transpose(1, 0, 2).astype(kdt)
        )
        in_maps.append({"data": dsh, "kern": ksh})
    return in_maps


def assemble(results) -> np.ndarray:
    out = np.empty((B, C, H, W), dtype=np.float32)
    for core in range(NCORES):
        b, hh = divmod(core, 2)
        out[b, :, hh * HS:(hh + 1) * HS, :] = results[core]["out"].transpose(1, 0, 2)
    return out


def kernel(data: np.ndarray, kernels: np.ndarray) -> np.ndarray:
    from concourse.bass_utils import run_bass_kernel_spmd

    nc = get_program()
    in_maps = make_shards(data, kernels)
    res = run_bass_kernel_spmd(nc, in_maps, list(range(NCORES)))
    return assemble(res.results)


if __name__ == "__main__":
    get_program()
    print("program built OK")


# revision 11
# speedup vs baseline: 1.0014x; 1.0014x over previous
"""Trainium2 Bass kernel: per-pixel 5x5 kernel application (KPN-style).

    out[b,c,y,x] = sum_{i,j} softmax(kernels[b,:,y,x])[i*5+j]
                   * zpad(data)[b,c,y+i,x+j]          (i,j in 0..4, r=2)

Sharding (8 NeuronCores, pure data parallel, no collectives):
    core = (b, H-half): 4 batches x 2 row-bands of 360 rows.
    Halo rows come from overlapping host-side slices of the full input.

Per-core algorithm (tiles live in "data space": 124 partitions =
120 output rows + 2 halo rows each side):
    - unnormalized softmax: E_t = exp(K_t) on ACT (bf16), denominator
      accumulated on the PE together with the taps.
    - DVE forms per-tap product planes Q_{t,c} = E_t * D_c (bf16, 2x mode);
      x-shift dj is a free-dim offset (two parity-aligned bf16 copies of the
      data keep operands 4-byte aligned); y-shift di is folded into the
      kernel-tensor DMA (rows loaded shifted by -di) and undone by the PE's
      stationary shift matrix S_di[k,m] = [k == m+di].
    - PE accumulates the 25 tap planes (and the 25 exp planes) into PSUM
      with shifted-identity matmuls.
    - DVE: out_c = PSUM_c * reciprocal(PSUM_sum).

DMA layout notes: kernel-tensor loads are one DMA per tap plane so the
DRAM side is a single contiguous region (strided small-chunk patterns get
pinned to a few SDMA engines); inputs are pre-converted to bf16 on the
host to halve HBM traffic; loads alternate between the two HWDGE rings
(sync + scalar); the output is staged full-width and stored once per
row-tile.

kernel(**inputs) takes the FULL inputs and returns the FULL output.
"""

import numpy as np
import ml_dtypes

B, C, H, W, KW = 4, 3, 720, 1280, 5
NCORES = 8
HS = H // 2            # 360 output rows per shard
RT = 120               # output rows per row-tile
NRT = HS // RT         # 3 row-tiles
HALO = 2
DP = RT + 2 * HALO     # 124 partitions (data space)
WP = 1288              # padded data width: 2 left + 1280 + 6 right
KROWPAD = 4            # zero rows around each kernel shard (top+bottom)
KH = HS + 2 * KROWPAD  # 368
XCH = [(0, 512), (512, 512), (1024, 256)]

KERN_BF16 = True       # ship kernels to HBM as bf16 (halves DMA traffic)

_CACHE = {}


def _build_program():
    import concourse.bacc as bacc
    import concourse.mybir as mybir
    from concourse.bass import AP
    from concourse import tile

    f32 = mybir.dt.float32
    bf16 = mybir.dt.bfloat16
    kdt = bf16 if KERN_BF16 else f32

    nc = bacc.Bacc(
        "TRN2",
        target_bir_lowering=False,
        debug=False,
        enable_asserts=False,
        num_devices=NCORES,
    )
    # Row-major host layouts: one large contiguous DMA descriptor per
    # SBUF partition (small strided descriptors throttle the SDMA engines).
    d_data = nc.dram_tensor("data", [HS + 2 * HALO, C, WP], bf16, kind="ExternalInput")
    d_kern = nc.dram_tensor("kern", [KH, KW * KW, W], kdt, kind="ExternalInput")
    d_out = nc.dram_tensor("out", [HS, C, W], f32, kind="ExternalOutput")

    # Shift matrices S_di[k, m] = 1 iff k == m + di  (k: 124 data rows,
    # m: 120 out rows). Baked into the NEFF as a Const tensor.
    s_np = np.zeros((KW, DP, RT), dtype=ml_dtypes.bfloat16)
    for di in range(KW):
        for m in range(RT):
            s_np[di, m + di, m] = 1.0
    d_s = nc.inline_tensor(np.ascontiguousarray(s_np), "smat")

    KROW = KW * KW * W  # element stride between rows of d_kern

    with tile.TileContext(nc) as tc:
        with tc.tile_pool(name="const", bufs=1) as cpool, \
             tc.tile_pool(name="dbf", bufs=2) as dbfpool, \
             tc.tile_pool(name="kt", bufs=3) as kpool, \
             tc.tile_pool(name="et", bufs=7) as epool, \
             tc.tile_pool(name="qt", bufs=4) as qpool, \
             tc.tile_pool(name="fin", bufs=2) as fpool, \
             tc.tile_pool(name="ps", bufs=2, space="PSUM") as ppool:

            s_sb = cpool.tile([DP, KW, RT], bf16)
            nc.sync.dma_start(out=s_sb[:], in_=d_s.ap().transpose([1, 0, 2]))

            for rt in range(NRT):
                y0 = rt * RT

                # data rows y0 .. y0+123 of the (row-padded) shard, bf16.
                # dbf1 is shifted one element left so odd-dj slices stay
                # 4-byte aligned (keeps DVE 2x mode).
                # kernel taps: one SWDGE DMA per di-group (5 tap planes,
                # rows shifted by -di) — SWDGE spreads descriptors across
                # all 16 SDMA engines (HWDGE pins them to 4); exp per group.
                # dbf0 also rides SWDGE (issued after the first kt so the
                # first exp is gated only by a 124-descriptor load); dbf1
                # (the 1-element x-shifted copy for odd-dj alignment) is a
                # cheap DVE bf16 copy, not a DMA, so no DMA->DMA dep chain.
                dbf0 = dbfpool.tile([DP, C, WP], bf16, tag="dbf0")
                dbf1 = dbfpool.tile([DP, C, WP], bf16, tag="dbf1")
                # byte-balance the three DMA paths (SWDGE sustains ~70GB/s
                # of descriptor flow; the two HWDGE rings share SDMA engines
                # 0-3 for ~110GB/s combined): di 0,1 + data on SWDGE,
                # di 2,3 on the sync ring, di 4 (+ the store) on scalar.
                ets = []
                kt_eng = [nc.sync, nc.scalar, nc.sync, nc.scalar, nc.gpsimd]
                for di in range(KW):
                    kt = kpool.tile([DP, KW, W], kdt, tag="kt")
                    off = (KROWPAD + y0 - di) * KROW + di * KW * W
                    kt_eng[di].dma_start(
                        out=kt[:],
                        in_=AP(d_kern, off, [[KROW, DP], [W, KW], [1, W]]),
                    )
                    if di == 0:
                        nc.gpsimd.dma_start(
                            out=dbf0[:], in_=d_data.ap()[y0:y0 + DP],
                        )
                    et = epool.tile([DP, KW, W], bf16, tag="et")
                    nc.scalar.activation(
                        et[:], kt[:], mybir.ActivationFunctionType.Exp,
                    )
                    ets.append(et)
                f0 = dbf0[:].rearrange("p c w -> p (c w)")
                f1 = dbf1[:].rearrange("p c w -> p (c w)")
                nc.vector.tensor_copy(f1[:, 0:C * WP - 1], f0[:, 1:C * WP])

                rs = fpool.tile([RT, W], f32, tag="rs", bufs=1)
                ost = fpool.tile([RT, C, W], f32, tag="ost")

                for (xc, xcw) in XCH:
                    # PSUM banks: 0..2 = channel accumulators, 3 = sumexp
                    pacc = ppool.tile([RT, 4, 512], f32, tag="pacc")

                    for di in range(KW):
                        et = ets[di]
                        lhs = s_sb[:, di, :]
                        first = di == 0
                        last = di == KW - 1
                        for dj in range(KW):
                            nc.tensor.matmul(
                                out=pacc[:, 3, 0:xcw],
                                lhsT=lhs,
                                rhs=et[:, dj, xc:xc + xcw],
                                start=first and dj == 0,
                                stop=last and dj == KW - 1,
                            )
                        for dj in range(KW):
                            qt = qpool.tile([DP, C, 512], bf16, tag="qt")
                            if dj % 2 == 0:
                                dsrc = dbf0[:, :, xc + dj:xc + dj + xcw]
                            else:
                                dsrc = dbf1[:, :, xc + dj - 1:xc + dj - 1 + xcw]
                            esrc = (
                                et[:, dj, xc:xc + xcw]
                                .unsqueeze(1)
                                .broadcast_to([DP, C, xcw])
                            )
                            nc.vector.tensor_tensor(
                                qt[:, :, 0:xcw], esrc, dsrc, mybir.AluOpType.mult,
                            )
                            for c in range(C):
                                nc.tensor.matmul(
                                    out=pacc[:, c, 0:xcw],
                                    lhsT=lhs,
                                    rhs=qt[:, c, 0:xcw],
                                    start=first and dj == 0,
                                    stop=last and dj == KW - 1,
                                )

                    nc.vector.reciprocal(rs[:, xc:xc + xcw], pacc[:, 3, 0:xcw])
                    rsb = (
                        rs[:, xc:xc + xcw].unsqueeze(1).broadcast_to([RT, C, xcw])
                    )
                    nc.vector.tensor_tensor(
                        ost[:, :, xc:xc + xcw], pacc[:, 0:3, 0:xcw], rsb,
                        mybir.AluOpType.mult,
                    )

                nc.gpsimd.dma_start(out=d_out.ap()[y0:y0 + RT], in_=ost[:])

    nc.compile()
    return nc


def get_program():
    if "nc" not in _CACHE:
        _CACHE["nc"] = _build_program()
    return _CACHE["nc"]


def make_shards(data: np.ndarray, kernels: np.ndarray):
    """Full inputs -> per-core input maps (with halo + zero padding)."""
    data = np.asarray(data, dtype=np.float32)
    kernels = np.asarray(kernels, dtype=np.float32)
    kdt = ml_dtypes.bfloat16 if KERN_BF16 else np.float32
    # zero-pad data: 2 rows top/bottom, 2 cols left, 6 cols right;
    # row-major layouts: data [row, c, x], kern [row, tap, x]
    dpad = np.zeros((B, H + 2 * HALO, C, WP), dtype=ml_dtypes.bfloat16)
    dpad[:, HALO:HALO + H, :, HALO:HALO + W] = (
        data.transpose(0, 2, 1, 3).astype(ml_dtypes.bfloat16)
    )
    in_maps = []
    for core in range(NCORES):
        b, hh = divmod(core, 2)
        r0 = hh * HS
        dsh = np.ascontiguousarray(dpad[b, r0:r0 + HS + 2 * HALO])
        ksh = np.zeros((KH, KW * KW, W), dtype=kdt)
        ksh[KROWPAD:KROWPAD + HS] = (
            kernels[b, :, r0:r0 + HS, :].transpose(1, 0, 2).astype(kdt)
        )
        in_maps.append({"data": dsh, "kern": ksh})
    return in_maps


def assemble(results) -> np.ndarray:
    out = np.empty((B, C, H, W), dtype=np.float32)
    for core in range(NCORES):
        b, hh = divmod(core, 2)
        out[b, :, hh * HS:(hh + 1) * HS, :] = results[core]["out"].transpose(1, 0, 2)
    return out


def kernel(data: np.ndarray, kernels: np.ndarray) -> np.ndarray:
    from concourse.bass_utils import run_bass_kernel_spmd

    nc = get_program()
    in_maps = make_shards(data, kernels)
    res = run_bass_kernel_spmd(nc, in_maps, list(range(NCORES)))
    return assemble(res.results)


if __name__ == "__main__":
    get_program()
    print("program built OK")


# revision 12
# speedup vs baseline: 1.0194x; 1.0180x over previous
"""Trainium2 Bass kernel: per-pixel 5x5 kernel application (KPN-style).

    out[b,c,y,x] = sum_{i,j} softmax(kernels[b,:,y,x])[i*5+j]
                   * zpad(data)[b,c,y+i,x+j]          (i,j in 0..4, r=2)

Sharding (8 NeuronCores, pure data parallel, no collectives):
    core = (b, H-half): 4 batches x 2 row-bands of 360 rows.
    Halo rows come from overlapping host-side slices of the full input.

Per-core algorithm (tiles live in "data space": 124 partitions =
120 output rows + 2 halo rows each side):
    - unnormalized softmax: E_t = exp(K_t) on ACT (bf16), denominator
      accumulated on the PE together with the taps.
    - DVE forms per-tap product planes Q_{t,c} = E_t * D_c (bf16, 2x mode);
      x-shift dj is a free-dim offset (two parity-aligned bf16 copies of the
      data keep operands 4-byte aligned); y-shift di is folded into the
      kernel-tensor DMA (rows loaded shifted by -di) and undone by the PE's
      stationary shift matrix S_di[k,m] = [k == m+di].
    - PE accumulates the 25 tap planes (and the 25 exp planes) into PSUM
      with shifted-identity matmuls.
    - DVE: out_c = PSUM_c * reciprocal(PSUM_sum).

DMA layout notes: kernel-tensor loads are one DMA per tap plane so the
DRAM side is a single contiguous region (strided small-chunk patterns get
pinned to a few SDMA engines); inputs are pre-converted to bf16 on the
host to halve HBM traffic; loads alternate between the two HWDGE rings
(sync + scalar); the output is staged full-width and stored once per
row-tile.

kernel(**inputs) takes the FULL inputs and returns the FULL output.
"""

import numpy as np
import ml_dtypes

B, C, H, W, KW = 4, 3, 720, 1280, 5
NCORES = 8
HS = H // 2            # 360 output rows per shard
RT = 120               # output rows per row-tile
NRT = HS // RT         # 3 row-tiles
HALO = 2
DP = RT + 2 * HALO     # 124 partitions (data space)
WP = 1288              # padded data width: 2 left + 1280 + 6 right
KROWPAD = 4            # zero rows around each kernel shard (top+bottom)
KH = HS + 2 * KROWPAD  # 368
XCH = [(0, 512), (512, 512), (1024, 256)]

KERN_BF16 = True       # ship kernels to HBM as bf16 (halves DMA traffic)

_CACHE = {}


def _build_program():
    import concourse.bacc as bacc
    import concourse.mybir as mybir
    from concourse.bass import AP
    from concourse import tile

    f32 = mybir.dt.float32
    bf16 = mybir.dt.bfloat16
    kdt = bf16 if KERN_BF16 else f32

    nc = bacc.Bacc(
        "TRN2",
        target_bir_lowering=False,
        debug=False,
        enable_asserts=False,
        num_devices=NCORES,
    )
    # Row-major host layouts: one large contiguous DMA descriptor per
    # SBUF partition (small strided descriptors throttle the SDMA engines).
    d_data = nc.dram_tensor("data", [HS + 2 * HALO, C, WP], bf16, kind="ExternalInput")
    d_kern = nc.dram_tensor("kern", [KH, KW * KW, W], kdt, kind="ExternalInput")
    d_out = nc.dram_tensor("out", [HS, C, W], f32, kind="ExternalOutput")

    # Shift matrices S_di[k, m] = 1 iff k == m + di  (k: 124 data rows,
    # m: 120 out rows). Baked into the NEFF as a Const tensor.
    s_np = np.zeros((KW, DP, RT), dtype=ml_dtypes.bfloat16)
    for di in range(KW):
        for m in range(RT):
            s_np[di, m + di, m] = 1.0
    d_s = nc.inline_tensor(np.ascontiguousarray(s_np), "smat")

    KROW = KW * KW * W  # element stride between rows of d_kern

    with tile.TileContext(nc) as tc:
        with tc.tile_pool(name="const", bufs=1) as cpool, \
             tc.tile_pool(name="dbf", bufs=2) as dbfpool, \
             tc.tile_pool(name="kt", bufs=3) as kpool, \
             tc.tile_pool(name="et", bufs=7) as epool, \
             tc.tile_pool(name="qt", bufs=4) as qpool, \
             tc.tile_pool(name="fin", bufs=2) as fpool, \
             tc.tile_pool(name="ps", bufs=2, space="PSUM") as ppool:

            s_sb = cpool.tile([DP, KW, RT], bf16)
            nc.sync.dma_start(out=s_sb[:], in_=d_s.ap().transpose([1, 0, 2]))

            pending_store = []

            def flush_store():
                while pending_store:
                    yy, t = pending_store.pop()
                    nc.gpsimd.dma_start(out=d_out.ap()[yy:yy + RT], in_=t[:])

            for rt in range(NRT):
                y0 = rt * RT

                # data rows y0 .. y0+123 of the (row-padded) shard, bf16.
                # dbf1 is shifted one element left so odd-dj slices stay
                # 4-byte aligned (keeps DVE 2x mode).
                # kernel taps: one SWDGE DMA per di-group (5 tap planes,
                # rows shifted by -di) — SWDGE spreads descriptors across
                # all 16 SDMA engines (HWDGE pins them to 4); exp per group.
                # dbf0 also rides SWDGE (issued after the first kt so the
                # first exp is gated only by a 124-descriptor load); dbf1
                # (the 1-element x-shifted copy for odd-dj alignment) is a
                # cheap DVE bf16 copy, not a DMA, so no DMA->DMA dep chain.
                dbf0 = dbfpool.tile([DP, C, WP], bf16, tag="dbf0")
                dbf1 = dbfpool.tile([DP, C, WP], bf16, tag="dbf1")
                # byte-balance the three DMA paths (SWDGE sustains ~70GB/s
                # of descriptor flow; the two HWDGE rings share SDMA engines
                # 0-3 for ~110GB/s combined): di 0,1 + data on SWDGE,
                # di 2,3 on the sync ring, di 4 (+ the store) on scalar.
                ets = []
                kt_eng = [nc.sync, nc.scalar, nc.sync, nc.scalar, nc.gpsimd]
                for di in range(KW):
                    kt = kpool.tile([DP, KW, W], kdt, tag="kt")
                    off = (KROWPAD + y0 - di) * KROW + di * KW * W
                    kt_eng[di].dma_start(
                        out=kt[:],
                        in_=AP(d_kern, off, [[KROW, DP], [W, KW], [1, W]]),
                    )
                    if di == 0:
                        nc.gpsimd.dma_start(
                            out=dbf0[:], in_=d_data.ap()[y0:y0 + DP],
                        )
                    et = epool.tile([DP, KW, W], bf16, tag="et")
                    nc.scalar.activation(
                        et[:], kt[:], mybir.ActivationFunctionType.Exp,
                    )
                    ets.append(et)
                f0 = dbf0[:].rearrange("p c w -> p (c w)")
                f1 = dbf1[:].rearrange("p c w -> p (c w)")
                nc.vector.tensor_copy(f1[:, 0:C * WP - 1], f0[:, 1:C * WP])
                flush_store()

                rs = fpool.tile([RT, W], f32, tag="rs", bufs=1)
                ost = fpool.tile([RT, C, W], f32, tag="ost")

                for (xc, xcw) in XCH:
                    # PSUM banks: 0..2 = channel accumulators, 3 = sumexp
                    pacc = ppool.tile([RT, 4, 512], f32, tag="pacc")

                    for di in range(KW):
                        et = ets[di]
                        lhs = s_sb[:, di, :]
                        first = di == 0
                        last = di == KW - 1
                        for dj in range(KW):
                            nc.tensor.matmul(
                                out=pacc[:, 3, 0:xcw],
                                lhsT=lhs,
                                rhs=et[:, dj, xc:xc + xcw],
                                start=first and dj == 0,
                                stop=last and dj == KW - 1,
                            )
                        for dj in range(KW):
                            qt = qpool.tile([DP, C, 512], bf16, tag="qt")
                            if dj % 2 == 0:
                                dsrc = dbf0[:, :, xc + dj:xc + dj + xcw]
                            else:
                                dsrc = dbf1[:, :, xc + dj - 1:xc + dj - 1 + xcw]
                            esrc = (
                                et[:, dj, xc:xc + xcw]
                                .unsqueeze(1)
                                .broadcast_to([DP, C, xcw])
                            )
                            nc.vector.tensor_tensor(
                                qt[:, :, 0:xcw], esrc, dsrc, mybir.AluOpType.mult,
                            )
                            for c in range(C):
                                nc.tensor.matmul(
                                    out=pacc[:, c, 0:xcw],
                                    lhsT=lhs,
                                    rhs=qt[:, c, 0:xcw],
                                    start=first and dj == 0,
                                    stop=last and dj == KW - 1,
                                )

                    nc.vector.reciprocal(rs[:, xc:xc + xcw], pacc[:, 3, 0:xcw])
                    rsb = (
                        rs[:, xc:xc + xcw].unsqueeze(1).broadcast_to([RT, C, xcw])
                    )
                    nc.vector.tensor_tensor(
                        ost[:, :, xc:xc + xcw], pacc[:, 0:3, 0:xcw], rsb,
                        mybir.AluOpType.mult,
                    )

                pending_store.append((y0, ost))

            flush_store()

    nc.compile()
    return nc


def get_program():
    if "nc" not in _CACHE:
        _CACHE["nc"] = _build_program()
    return _CACHE["nc"]


def make_shards(data: np.ndarray, kernels: np.ndarray):
    """Full inputs -> per-core input maps (with halo + zero padding)."""
    data = np.asarray(data, dtype=np.float32)
    kernels = np.asarray(kernels, dtype=np.float32)
    kdt = ml_dtypes.bfloat16 if KERN_BF16 else np.float32
    # zero-pad data: 2 rows top/bottom, 2 cols left, 6 cols right;
    # row-major layouts: data [row, c, x], kern [row, tap, x]
    dpad = np.zeros((B, H + 2 * HALO, C, WP), dtype=ml_dtypes.bfloat16)
    dpad[:, HALO:HALO + H, :, HALO:HALO + W] = (
        data.transpose(0, 2, 1, 3).astype(ml_dtypes.bfloat16)
    )
    in_maps = []
    for core in range(NCORES):
        b, hh = divmod(core, 2)
        r0 = hh * HS
        dsh = np.ascontiguousarray(dpad[b, r0:r0 + HS + 2 * HALO])
        ksh = np.zeros((KH, KW * KW, W), dtype=kdt)
        ksh[KROWPAD:KROWPAD + HS] = (
            kernels[b, :, r0:r0 + HS, :].transpose(1, 0, 2).astype(kdt)
        )
        in_maps.append({"data": dsh, "kern": ksh})
    return in_maps


def assemble(results) -> np.ndarray:
    out = np.empty((B, C, H, W), dtype=np.float32)
    for core in range(NCORES):
        b, hh = divmod(core, 2)
        out[b, :, hh * HS:(hh + 1) * HS, :] = results[core]["out"].transpose(1, 0, 2)
    return out


def kernel(data: np.ndarray, kernels: np.ndarray) -> np.ndarray:
    from concourse.bass_utils import run_bass_kernel_spmd

    nc = get_program()
    in_maps = make_shards(data, kernels)
    res = run_bass_kernel_spmd(nc, in_maps, list(range(NCORES)))
    return assemble(res.results)


if __name__ == "__main__":
    get_program()
    print("program built OK")


# revision 13
# speedup vs baseline: 1.0433x; 1.0234x over previous
"""Trainium2 Bass kernel: per-pixel 5x5 kernel application (KPN-style).

    out[b,c,y,x] = sum_{i,j} softmax(kernels[b,:,y,x])[i*5+j]
                   * zpad(data)[b,c,y+i,x+j]          (i,j in 0..4, r=2)

Sharding (8 NeuronCores, pure data parallel, no collectives):
    core = (b, H-half): 4 batches x 2 row-bands of 360 rows.
    Halo rows come from overlapping host-side slices of the full input.

Per-core algorithm (tiles live in "data space": 124 partitions =
120 output rows + 2 halo rows each side):
    - unnormalized softmax: E_t = exp(K_t) on ACT (bf16), denominator
      accumulated on the PE together with the taps.
    - DVE forms per-tap product planes Q_{t,c} = E_t * D_c (bf16, 2x mode);
      x-shift dj is a free-dim offset (two parity-aligned bf16 copies of the
      data keep operands 4-byte aligned); y-shift di is folded into the
      kernel-tensor DMA (rows loaded shifted by -di) and undone by the PE's
      stationary shift matrix S_di[k,m] = [k == m+di].
    - PE accumulates the 25 tap planes (and the 25 exp planes) into PSUM
      with shifted-identity matmuls.
    - DVE: out_c = PSUM_c * reciprocal(PSUM_sum).

DMA layout notes: kernel-tensor loads are one DMA per tap plane so the
DRAM side is a single contiguous region (strided small-chunk patterns get
pinned to a few SDMA engines); inputs are pre-converted to bf16 on the
host to halve HBM traffic; loads alternate between the two HWDGE rings
(sync + scalar); the output is staged full-width and stored once per
row-tile.

kernel(**inputs) takes the FULL inputs and returns the FULL output.
"""

import numpy as np
import ml_dtypes

B, C, H, W, KW = 4, 3, 720, 1280, 5
NCORES = 8
HS = H // 2            # 360 output rows per shard
RT = 120               # output rows per row-tile
NRT = HS // RT         # 3 row-tiles
HALO = 2
DP = RT + 2 * HALO     # 124 partitions (data space)
WP = 1288              # padded data width: 2 left + 1280 + 6 right
KROWPAD = 4            # zero rows around each kernel shard (top+bottom)
KH = HS + 2 * KROWPAD  # 368
XCH = [(0, 512), (512, 512), (1024, 256)]

KERN_BF16 = True       # ship kernels to HBM as bf16 (halves DMA traffic)

_CACHE = {}


def _build_program():
    import concourse.bacc as bacc
    import concourse.mybir as mybir
    from concourse.bass import AP
    from concourse import tile

    f32 = mybir.dt.float32
    bf16 = mybir.dt.bfloat16
    kdt = bf16 if KERN_BF16 else f32

    nc = bacc.Bacc(
        "TRN2",
        target_bir_lowering=False,
        debug=False,
        enable_asserts=False,
        num_devices=NCORES,
    )
    # Row-major host layouts: one large contiguous DMA descriptor per
    # SBUF partition (small strided descriptors throttle the SDMA engines).
    d_data = nc.dram_tensor("data", [HS + 2 * HALO, C, WP], bf16, kind="ExternalInput")
    d_kern = nc.dram_tensor("kern", [KH, KW * KW, W], kdt, kind="ExternalInput")
    d_out = nc.dram_tensor("out", [HS, C, W], f32, kind="ExternalOutput")

    # Shift matrices S_di[k, m] = 1 iff k == m + di  (k: 124 data rows,
    # m: 120 out rows). Baked into the NEFF as a Const tensor.
    s_np = np.zeros((KW, DP, RT), dtype=ml_dtypes.bfloat16)
    for di in range(KW):
        for m in range(RT):
            s_np[di, m + di, m] = 1.0
    d_s = nc.inline_tensor(np.ascontiguousarray(s_np), "smat")

    KROW = KW * KW * W  # element stride between rows of d_kern

    with tile.TileContext(nc) as tc:
        with tc.tile_pool(name="const", bufs=1) as cpool, \
             tc.tile_pool(name="dbf", bufs=2) as dbfpool, \
             tc.tile_pool(name="kt", bufs=3) as kpool, \
             tc.tile_pool(name="et", bufs=7) as epool, \
             tc.tile_pool(name="qt", bufs=4) as qpool, \
             tc.tile_pool(name="fin", bufs=2) as fpool, \
             tc.tile_pool(name="ps", bufs=2, space="PSUM") as ppool:

            s_sb = cpool.tile([DP, KW, RT], bf16)
            nc.sync.dma_start(out=s_sb[:], in_=d_s.ap().transpose([1, 0, 2]))

            pending_store = []

            def flush_store():
                while pending_store:
                    yy, t = pending_store.pop()
                    nc.gpsimd.dma_start(out=d_out.ap()[yy:yy + RT], in_=t[:])

            for rt in range(NRT):
                y0 = rt * RT

                # data rows y0 .. y0+123 of the (row-padded) shard, bf16.
                # dbf1 is shifted one element left so odd-dj slices stay
                # 4-byte aligned (keeps DVE 2x mode).
                # kernel taps: one SWDGE DMA per di-group (5 tap planes,
                # rows shifted by -di) — SWDGE spreads descriptors across
                # all 16 SDMA engines (HWDGE pins them to 4); exp per group.
                # dbf0 also rides SWDGE (issued after the first kt so the
                # first exp is gated only by a 124-descriptor load); dbf1
                # (the 1-element x-shifted copy for odd-dj alignment) is a
                # cheap DVE bf16 copy, not a DMA, so no DMA->DMA dep chain.
                dbf0 = dbfpool.tile([DP, C, WP], bf16, tag="dbf0")
                dbf1 = dbfpool.tile([DP, C, WP], bf16, tag="dbf1")
                # byte-balance the three DMA paths (SWDGE sustains ~70GB/s
                # of descriptor flow; the two HWDGE rings share SDMA engines
                # 0-3 for ~110GB/s combined): di 0,1 + data on SWDGE,
                # di 2,3 on the sync ring, di 4 (+ the store) on scalar.
                ets = []
                kt_eng = [nc.sync, nc.scalar, nc.sync, nc.scalar, nc.gpsimd]
                for di in range(KW):
                    kt = kpool.tile([DP, KW, W], kdt, tag="kt")
                    et = epool.tile([DP, KW, W], bf16, tag="et")
                    for dj in range(KW):
                        off = (KROWPAD + y0 - di) * KROW + (di * KW + dj) * W
                        kt_eng[di].dma_start(
                            out=kt[:, dj, :],
                            in_=AP(d_kern, off, [[KROW, DP], [1, W]]),
                        )
                        nc.scalar.activation(
                            et[:, dj, :], kt[:, dj, :],
                            mybir.ActivationFunctionType.Exp,
                        )
                    if di == 0:
                        nc.gpsimd.dma_start(
                            out=dbf0[:], in_=d_data.ap()[y0:y0 + DP],
                        )
                    ets.append(et)
                f0 = dbf0[:].rearrange("p c w -> p (c w)")
                f1 = dbf1[:].rearrange("p c w -> p (c w)")
                nc.vector.tensor_copy(f1[:, 0:C * WP - 1], f0[:, 1:C * WP])
                flush_store()

                rs = fpool.tile([RT, W], f32, tag="rs", bufs=1)
                ost = fpool.tile([RT, C, W], f32, tag="ost")

                for (xc, xcw) in XCH:
                    # PSUM banks: 0..2 = channel accumulators, 3 = sumexp
                    pacc = ppool.tile([RT, 4, 512], f32, tag="pacc")

                    for di in range(KW):
                        et = ets[di]
                        lhs = s_sb[:, di, :]
                        first = di == 0
                        last = di == KW - 1
                        for dj in range(KW):
                            nc.tensor.matmul(
                                out=pacc[:, 3, 0:xcw],
                                lhsT=lhs,
                                rhs=et[:, dj, xc:xc + xcw],
                                start=first and dj == 0,
                                stop=last and dj == KW - 1,
                            )
                        for dj in range(KW):
                            qt = qpool.tile([DP, C, 512], bf16, tag="qt")
                            if dj % 2 == 0:
                                dsrc = dbf0[:, :, xc + dj:xc + dj + xcw]
                            else:
                                dsrc = dbf1[:, :, xc + dj - 1:xc + dj - 1 + xcw]
                            esrc = (
                                et[:, dj, xc:xc + xcw]
                                .unsqueeze(1)
                                .broadcast_to([DP, C, xcw])
                            )
                            nc.vector.tensor_tensor(
                                qt[:, :, 0:xcw], esrc, dsrc, mybir.AluOpType.mult,
                            )
                            for c in range(C):
                                nc.tensor.matmul(
                                    out=pacc[:, c, 0:xcw],
                                    lhsT=lhs,
                                    rhs=qt[:, c, 0:xcw],
                                    start=first and dj == 0,
                                    stop=last and dj == KW - 1,
                                )

                    nc.vector.reciprocal(rs[:, xc:xc + xcw], pacc[:, 3, 0:xcw])
                    rsb = (
                        rs[:, xc:xc + xcw].unsqueeze(1).broadcast_to([RT, C, xcw])
                    )
                    nc.vector.tensor_tensor(
                        ost[:, :, xc:xc + xcw], pacc[:, 0:3, 0:xcw], rsb,
                        mybir.AluOpType.mult,
                    )

                pending_store.append((y0, ost))

            flush_store()

    nc.compile()
    return nc


def get_program():
    if "nc" not in _CACHE:
        _CACHE["nc"] = _build_program()
    return _CACHE["nc"]


def make_shards(data: np.ndarray, kernels: np.ndarray):
    """Full inputs -> per-core input maps (with halo + zero padding)."""
    data = np.asarray(data, dtype=np.float32)
    kernels = np.asarray(kernels, dtype=np.float32)
    kdt = ml_dtypes.bfloat16 if KERN_BF16 else np.float32
    # zero-pad data: 2 rows top/bottom, 2 cols left, 6 cols right;
    # row-major layouts: data [row, c, x], kern [row, tap, x]
    dpad = np.zeros((B, H + 2 * HALO, C, WP), dtype=ml_dtypes.bfloat16)
    dpad[:, HALO:HALO + H, :, HALO:HALO + W] = (
        data.transpose(0, 2, 1, 3).astype(ml_dtypes.bfloat16)
    )
    in_maps = []
    for core in range(NCORES):
        b, hh = divmod(core, 2)
        r0 = hh * HS
        dsh = np.ascontiguousarray(dpad[b, r0:r0 + HS + 2 * HALO])
        ksh = np.zeros((KH, KW * KW, W), dtype=kdt)
        ksh[KROWPAD:KROWPAD + HS] = (
            kernels[b, :, r0:r0 + HS, :].transpose(1, 0, 2).astype(kdt)
        )
        in_maps.append({"data": dsh, "kern": ksh})
    return in_maps


def assemble(results) -> np.ndarray:
    out = np.empty((B, C, H, W), dtype=np.float32)
    for core in range(NCORES):
        b, hh = divmod(core, 2)
        out[b, :, hh * HS:(hh + 1) * HS, :] = results[core]["out"].transpose(1, 0, 2)
    return out


def kernel(data: np.ndarray, kernels: np.ndarray) -> np.ndarray:
    from concourse.bass_utils import run_bass_kernel_spmd

    nc = get_program()
    in_maps = make_shards(data, kernels)
    res = run_bass_kernel_spmd(nc, in_maps, list(range(NCORES)))
    return assemble(res.results)


if __name__ == "__main__":
    get_program()
    print("program built OK")
